# revision 31
# baseline (speedup 1.0000x reference)
"""Trainium2 Bass kernel for nn_Attention_59347858096503 (v2).

Reference computation (per batch b, head H):
    k = split_heads(key_in @ Wk + bk); q = ...; v = ...
    qsum = q.sum(axis=positions)                      # (b,H,D)
    scores[s] = k[s] . qsum                           # per-key score (no q dep!)
    attn[q,k] = softmax over keys k<=q of scores[k]   # prefix softmax
    ctx[q] = sum_k attn[q,k] v[k];  out = ctx @ Wo + bo

Attention reduces to a prefix-softmax-weighted running average of v computed
blockwise (128 keys/queries per block) with a running (N, Z) carry at the
prefix-max scale m (prefix max of scores).

v2 design vs v1:
  - fp16 host staging of every tensor (score chain float accumulates on
    device stay f32/PSUM): halves HBM traffic; PE matmuls hit 1 cy/row.
  - G = mask + m[t] - s[k] built by PE rank-1 matmuls (mask via identity
    contraction, [ones; scoresT] stacked A against [m; -blockdiag] B),
    consumed by ACT exp directly from PSUM -> F fp16 in SBUF.
  - CT produced TRANSPOSED: matmul(CT[d,t], V[k,d-head], F[k,t]) so ctx
    needs no PE transposes / PSUM round trips before the out-projection.
  - Z rides V as a 65th ones-column (Vaug); per-block per-head carry rows
    NZ = [N | Z] at scale m[last] come from F's last column in 2 matmuls.
  - alpha (carry rescale) = one ACT exp per block on [8,128] with
    per-partition bias.
  - 1/Z broadcast to ctx layout via PE (indg x zrT), ctx = CT * zrb in one
    DVE op per block writing fp16 ctxT.
  - out-proj reads ctxT tiles directly; results DMA straight from PSUM to
    HBM (f32), skipping oT SBUF copies.

Sharding: batch 4-way x head-half 2-way = 8 cores; host sums the two f32
partials per batch. bk drops (softmax shift); bq via qsum; bv/bo on host.
"""
import os
import sys

sys.path.insert(0, "/opt/trn_rl_repo")

import numpy as np
import concourse.bass as bass
import concourse.tile as tile
from concourse import bacc, mybir
from concourse.bass_utils import run_bass_kernel_spmd

F32 = mybir.dt.float32
F32R = mybir.dt.float32r
F16 = mybir.dt.float16
AF = mybir.ActivationFunctionType
ALU = mybir.AluOpType

S = 1024
D = 1024
NH = 8        # heads per core
HD = 64       # head dim
NB = 8        # key/query blocks of 128
NC = 8        # cores
BIG = 30000.0


def _r(ap):
    return ap.bitcast(F32R)


def build(num_devices=NC):
    nc = bacc.Bacc(None, target_bir_lowering=False, debug=False,
                   num_devices=num_devices)

    xq_d = nc.dram_tensor("xq", [S, D], F16, kind="ExternalInput")
    xkT_d = nc.dram_tensor("xkT", [D, S], F16, kind="ExternalInput")
    xvT_d = nc.dram_tensor("xvT", [D, S], F16, kind="ExternalInput")
    wq_d = nc.dram_tensor("wq", [D, 512], F16, kind="ExternalInput")
    wkT_d = nc.dram_tensor("wkT", [512, D], F16, kind="ExternalInput")
    wv_d = nc.dram_tensor("wv", [D, 512], F16, kind="ExternalInput")
    wo_d = nc.dram_tensor("wo", [512, D], F16, kind="ExternalInput")
    qb_d = nc.dram_tensor("qbT", [128, 4], F32, kind="ExternalInput")
    mask_d = nc.dram_tensor("masktile", [128, 512], F32, kind="ExternalInput")
    id_d = nc.dram_tensor("ident", [128, 128], F32, kind="ExternalInput")
    bd_d = nc.dram_tensor("bdmask", [128, 4, 8], F32, kind="ExternalInput")
    nd_d = nc.dram_tensor("negdiag", [8, 1024], F32, kind="ExternalInput")
    ig_d = nc.dram_tensor("indg", [8, 512], F16, kind="ExternalInput")
    outT_d = nc.dram_tensor("outT", [D, S], F16, kind="ExternalOutput")
    dbg_qt_d = nc.dram_tensor("dbg_qt", [128, 4], F32, kind="ExternalOutput")
    dbg_xqs_d = nc.dram_tensor("dbg_xqs", [128, 8], F16, kind="ExternalOutput")

    with tile.TileContext(nc) as tc:
        with (
            tc.tile_pool(name="const", bufs=1) as cpool,
            tc.tile_pool(name="wts", bufs=1) as wpool,
            tc.tile_pool(name="big", bufs=1) as bpool,
            tc.tile_pool(name="sc", bufs=1) as scpool,
            tc.tile_pool(name="xs", bufs=3) as xspool,
            tc.tile_pool(name="sm", bufs=2) as smpool,
            tc.tile_pool(name="psG", bufs=2, space="PSUM") as psG,
            tc.tile_pool(name="psC1", bufs=1, space="PSUM") as psC1,
            tc.tile_pool(name="psC2", bufs=2, space="PSUM") as psC2,
            tc.tile_pool(name="psZ", bufs=2, space="PSUM") as psZ,
            tc.tile_pool(name="psM", bufs=1, space="PSUM") as psM,
        ):
            # ---- constants ----
            ident = cpool.tile([128, 128], F32)
            nc.sync.dma_start(ident[:], id_d.ap())
            identr = cpool.tile([128, 128], F32R)
            nc.gpsimd.dma_start(out=identr[:], in_=id_d.ap())
            masktile = cpool.tile([128, 512], F32R)
            nc.gpsimd.dma_start(out=masktile[:], in_=mask_d.ap())
            negdiag = cpool.tile([8, 1024], F32R)
            nc.gpsimd.dma_start(out=negdiag[:], in_=nd_d.ap())
            indg = cpool.tile([8, 512], F16)
            nc.sync.dma_start(indg[:], ig_d.ap())
            qb_sb = cpool.tile([128, 4], F32)
            nc.sync.dma_start(qb_sb[:], qb_d.ap())
            bdmask = cpool.tile([128, 4, 8], F32)
            nc.sync.dma_start(bdmask[:], bd_d.ap())
            ones_col = cpool.tile([128, 1], F16)
            nc.vector.memset(ones_col[:], 1.0)
            ones_f32 = cpool.tile([1, 128], F32)
            nc.vector.memset(ones_f32[:], 1.0)
            ones_row = cpool.tile([1, 128], F32R)
            nc.gpsimd.dma_start(out=ones_row[:], in_=ones_f32[:])

            # m_flat[0, b, :] = per-block m rows (flattened by SWDGE later)
            m_flat = scpool.tile([1, NB, 1024], F32R, name="m_flat")

            # ---- weights for the score chain first (critical path) ----
            wq_sb = wpool.tile([128, 8, 512], F16)
            nc.sync.dma_start(wq_sb[:], wq_d.ap().rearrange("(j p) f -> p j f", p=128))

            # ---- xq stream (s-major) -> PE column sums xqsumT[f,fc] ----
            xqsumT_ps = psM.tile([128, 512], F32, tag="m1", name="xqsumT_ps")[:, 0:8]
            xts = []
            for j in range(8):
                xt = xspool.tile([128, 1024], F16, tag="xq", bufs=8)
                nc.sync.dma_start(xt[:], xq_d.ap()[j * 128:(j + 1) * 128, :])
                xts.append(xt)
            # consecutive matmuls per accumulation group (interleaved open
            # groups in one PSUM bank lose writes)
            for fc in range(8):
                for j in range(8):
                    nc.tensor.matmul(xqsumT_ps[:, fc:fc + 1],
                                     xts[j][:, fc * 128:(fc + 1) * 128], ones_col[:],
                                     start=(j == 0), stop=(j == 7))
            xqsumT = scpool.tile([128, 8], F16)
            nc.vector.tensor_copy(xqsumT[:], xqsumT_ps[:])

            xv_ap = xvT_d.ap().rearrange("(j p) s -> p j s", p=128)

            # ---- qsumT[dm, c] columns = sum_f wq[f, dm] xqsum[f] + S*bq ----
            qsumT_ps = psM.tile([128, 512], F32, tag="m1", name="qsumT_ps")[:, 0:4]
            for dc in range(4):
                for fc in range(8):
                    nc.tensor.matmul(qsumT_ps[:, dc:dc + 1],
                                     wq_sb[:, fc, dc * 128:(dc + 1) * 128],
                                     xqsumT[:, fc:fc + 1],
                                     start=(fc == 0), stop=(fc == 7))
            qt_sb = scpool.tile([128, 4], F32)
            nc.vector.tensor_add(qt_sb[:], qsumT_ps[:], qb_sb[:])

            nc.sync.dma_start(dbg_qt_d.ap(), qt_sb[:])
            nc.sync.dma_start(dbg_xqs_d.ap(), xqsumT[:])
            # ---- qsblk[f, c, h] = qsum[c*128+f] * (head(c,f) == h), fp16 ----
            qsblk = scpool.tile([128, 4, 8], F16)
            for c in range(4):
                nc.vector.tensor_scalar_mul(qsblk[:, c, :], bdmask[:, c, :],
                                            qt_sb[:, c:c + 1])

            wkT_sb = wpool.tile([128, 4, 1024], F16)
            nc.sync.dma_start(wkT_sb[:], wkT_d.ap().rearrange("(c p) m -> p c m", p=128))

            # ---- u[dm, h] = sum_f wkT[f, dm] qsblk[f, h] ----
            u_sb = scpool.tile([128, 8, 8], F16)
            for i in range(8):
                u_ps = psM.tile([128, 512], F32, tag="m1", name="u_ps")[:, 0:8]
                for c in range(4):
                    nc.tensor.matmul(u_ps[:], wkT_sb[:, c, i * 128:(i + 1) * 128],
                                     qsblk[:, c, :], start=(c == 0), stop=(c == 3))
                nc.vector.tensor_copy(u_sb[:, i, :], u_ps[:])

            # ---- scoresT (8, 1024) = u.T @ xkT (xk streamed) ----
            scoresT_psA = psG.tile([128, 512], F32, tag="g", name="scoresT_psA")[0:8, :]
            scoresT_psB = psG.tile([128, 512], F32, tag="g", name="scoresT_psB")[0:8, :]
            sc_half = (scoresT_psA, scoresT_psB)
            for i in range(8):
                xt = xspool.tile([128, 1024], F16, tag="xk", bufs=4)
                nc.sync.dma_start(xt[:], xkT_d.ap()[i * 128:(i + 1) * 128, :])
                for half in range(2):
                    nc.tensor.matmul(
                        sc_half[half][:],
                        u_sb[:, i, :], xt[:, half * 512:(half + 1) * 512],
                        start=(i == 0), stop=(i == 7))

            # ---- V-chain loads AFTER the xk stream on the same (sync) queue:
            # the score chain owns the DMA pipe first; V-proj JIT eats slices.
            wv_sb = wpool.tile([128, 8, 512], F16)
            xvT_sb = bpool.tile([128, 8, 1024], F16, name="xvT_sb")
            nc.sync.dma_start(wv_sb[:], wv_d.ap().rearrange("(j p) f -> p j f", p=128))
            for vs in range(4):
                nc.sync.dma_start(xvT_sb[:, :, vs * 256:(vs + 1) * 256],
                                  xv_ap[:, :, vs * 256:(vs + 1) * 256])
            wo_sb = wpool.tile([128, 4, 1024], F16)
            nc.sync.dma_start(wo_sb[:], wo_d.ap().rearrange("(c p) m -> p c m", p=128))

            # ---- scoresT rows in SBUF (lhsT for the G s-broadcast) ----
            scA = scpool.tile([8, 1024], F32R, name="scA")
            nc.vector.tensor_copy(scA[:, 0:512], scoresT_psA[:])
            nc.vector.tensor_copy(scA[:, 512:1024], scoresT_psB[:])

            # ---- chained prefix-max m per block + mS row DMAs + alpha ----
            m_all = scpool.tile([8, 1024], F32R, name="m_all")
            alpha8 = scpool.tile([8, NB, 128], F16, name="alpha8")
            alpha = scpool.tile([1, NB, 1024], F16, name="alpha")
            for blk in range(NB):
                sl = slice(blk * 128, (blk + 1) * 128)
                init = -3.0e38 if blk == 0 else m_all[:, blk * 128 - 1:blk * 128]
                nc.vector.tensor_tensor_scan(m_all[:, sl], scA[:, sl],
                                             scA[:, sl], init,
                                             ALU.max, ALU.max)
                nc.gpsimd.dma_start(out=m_flat[0:1, blk, :], in_=m_all[:, sl])
                if blk > 0:
                    prev = blk * 128 - 1
                    nc.scalar.activation(alpha8[:, blk, :], m_all[:, sl],
                                         AF.Exp, scale=-1.0,
                                         bias=m_all[:, prev:prev + 1])
                    nc.gpsimd.dma_start(out=alpha[0:1, blk, :], in_=alpha8[:, blk, :])

            # ---- V_sb: per-block projected V with a ones 65th column ----
            V_sb = bpool.tile([128, NB, NH, 65], F16, name="V_sb")
            nc.vector.memset(V_sb[:], 1.0)

            def emit_vproj(blk):
                V_ps = psZ.tile([128, 512], F32, tag="z", name="V_ps")
                for j in range(8):
                    nc.tensor.matmul(
                        V_ps[:],
                        xvT_sb[:, j, blk * 128:(blk + 1) * 128],
                        wv_sb[:, j, :],
                        start=(j == 0), stop=(j == 7))
                if blk % 2 == 0:
                    nc.vector.tensor_copy(V_sb[:, blk, :, 0:64], V_ps[:])
                else:
                    nc.scalar.copy(V_sb[:, blk, :, 0:64], V_ps[:])

            # ---- attention blocks ----
            F_sb = bpool.tile([128, 2, 1024], F16, name="F_sb")
            ctxT_sb = bpool.tile([128, 4, 1024], F16, name="ctxT_sb")
            NZprev = None

            def emit_pair(q):
                # out-proj for s-blocks 2q, 2q+1 -> outT[j, q*256:(q+1)*256]
                ssl = slice(q * 256, (q + 1) * 256)
                oT = smpool.tile([128, 8, 256], F16, tag="oT", name="oT", bufs=2)
                for jj in range(4):
                    O_ps = psZ.tile([128, 512], F32, tag="z", name="O_ps")
                    for j2 in range(2):
                        jb = jj * 2 + j2
                        for c in range(4):
                            nc.tensor.matmul(
                                O_ps[:, j2 * 256:(j2 + 1) * 256],
                                wo_sb[:, c, jb * 128:(jb + 1) * 128],
                                ctxT_sb[:, c, ssl],
                                start=(c == 0), stop=(c == 3))
                    dst = oT[:, jj * 2:jj * 2 + 2, :]
                    srcv = O_ps[:].rearrange("p (j2 s) -> p j2 s", j2=2)
                    if jj % 2 == 0:
                        nc.vector.tensor_copy(dst, srcv)
                    else:
                        nc.scalar.copy(dst, srcv)
                nc.sync.dma_start(
                    outT_d.ap().rearrange("(jb p) s -> p jb s", p=128)[:, :, ssl],
                    oT[:])

            for blk in range(NB):
                sl = slice(blk * 128, (blk + 1) * 128)
                par = blk % 2
                if blk == 0:
                    emit_vproj(0)
                    emit_vproj(1)
                elif blk < NB - 1:
                    emit_vproj(blk + 1)
                # --- G = mask + m[t] - s[k] per half; F = exp(-G) fp16 ---
                G_h = []
                for half in range(2):
                    hsl = slice(half * 512, (half + 1) * 512)
                    G_ps = psG.tile([128, 512], F32, tag="g", name="G_ps")
                    nc.tensor.matmul(G_ps[:], identr[:], masktile[:],
                                     start=True, stop=False)
                    nc.tensor.matmul(G_ps[:], ones_row[:],
                                     m_flat[0:1, blk, hsl],
                                     start=False, stop=False)
                    nc.tensor.matmul(G_ps[:], scA[:, sl],
                                     negdiag[:, hsl],
                                     start=False, stop=True)
                    G_h.append(G_ps)
                # carries into CT/CZ first (ready before F) then F-dependent.
                # CZNZ bank: CZ [128 t, 8 h] at cols 0:8; NZ rows (N|Z at
                # scale m[last]) at partitions 0 (heads 0-3) / 64 (heads 4-7),
                # cols 128:388.
                CZNZ = psC1.tile([128, 512], F32, tag="cz", name="CZNZ")
                CZ = CZNZ[:, 0:8]
                CT = psC2.tile([128, 512], F32, tag="ct", name="CT")

                def nz_view(t, h):
                    p0 = 64 * (h // 4)
                    c0 = 128 + (h % 4) * 65
                    return t[p0:p0 + 1, c0:c0 + 65]

                for half in range(2):
                    hsl = slice(half * 512, (half + 1) * 512)
                    nc.scalar.activation(F_sb[:, par, hsl], G_h[half][:],
                                         AF.Exp, scale=-1.0)
                first = blk == 0
                for h in range(NH):
                    g, dpar = h // 2, h % 2
                    Fh = F_sb[:, par, h * 128:(h + 1) * 128]
                    nc.tensor.matmul(
                        CT[dpar * 64:(dpar + 1) * 64, g * 128:(g + 1) * 128],
                        V_sb[:, blk, h, 0:64], Fh,
                        start=True, stop=first)
                    nc.tensor.matmul(
                        CZ[:, h:h + 1], Fh, ones_col[:],
                        start=True, stop=first)
                    if blk > 0:
                        nc.tensor.matmul(
                            CT[dpar * 64:(dpar + 1) * 64, g * 128:(g + 1) * 128],
                            NZprev[0:1, (h // 4) * 260 + (h % 4) * 65:(h // 4) * 260 + (h % 4) * 65 + 64],
                            alpha[0:1, blk, h * 128:(h + 1) * 128],
                            start=False, stop=True)
                        nc.tensor.matmul(
                            CZ[:, h:h + 1],
                            alpha[0:1, blk, h * 128:(h + 1) * 128],
                            NZprev[0:1, (h // 4) * 260 + (h % 4) * 65 + 64:(h // 4) * 260 + (h % 4) * 65 + 65],
                            start=False, stop=True)
                    if blk < NB - 1:
                        nc.tensor.matmul(
                            nz_view(CZNZ, h),
                            F_sb[:, par, h * 128 + 127:h * 128 + 128],
                            V_sb[:, blk, h, 0:65],
                            start=True, stop=first)
                        if blk > 0:
                            nc.tensor.matmul(
                                nz_view(CZNZ, h),
                                alpha[0:1, blk, h * 128 + 127:h * 128 + 128],
                                NZprev[0:1, (h // 4) * 260 + (h % 4) * 65:(h // 4) * 260 + (h % 4) * 65 + 65],
                                start=False, stop=True)
                # 1/Z [128,8] -> PE transpose -> zrT [8,128] fp16
                zr_all = smpool.tile([128, 8], F32, tag="zra", name="zr_all")
                nc.vector.reciprocal(zr_all[:], CZ[:])
                zrt_ps = psM.tile([128, 512], F32, tag="m1", name="zrt_ps")
                nc.tensor.transpose(zrt_ps[0:8, 0:128], zr_all[:], ident[:])
                zrT = smpool.tile([8, 128], F16, tag="zrt", name="zrT")
                nc.vector.tensor_copy(zrT[:], zrt_ps[0:8, 0:128])
                if blk < NB - 1:
                    NZnew = smpool.tile([1, 520], F16, tag="nzsb", name="NZsb")
                    nc.vector.tensor_copy(NZnew[0:1, 0:260], CZNZ[0:1, 128:388])
                    nc.vector.tensor_copy(NZnew[0:1, 260:520], CZNZ[64:65, 128:388])
                    NZprev = NZnew
                zrb = psM.tile([128, 512], F32, tag="m1", name="zrb")
                for g in range(4):
                    nc.tensor.matmul(zrb[:, g * 128:(g + 1) * 128],
                                     indg[:, g * 128:(g + 1) * 128],
                                     zrT[:], start=True, stop=True)
                zrb_sb = smpool.tile([128, 512], F16, tag="zrb_sb", name="zrb_sb")
                nc.scalar.copy(zrb_sb[:], zrb[:])
                nc.vector.tensor_tensor(
                    ctxT_sb[:, :, sl],
                    CT[:].rearrange("p (c s) -> p c s", c=4),
                    zrb_sb[:].rearrange("p (c s) -> p c s", c=4), ALU.mult)

                if blk % 2 == 1:
                    emit_pair(blk // 2)
            


    nc.compile()
    return nc


_NC_CACHE = {}


def _get_nc():
    if "nc" not in _NC_CACHE:
        _NC_CACHE["nc"] = build()
    return _NC_CACHE["nc"]


def _consts():
    p = np.arange(128)
    masktri = np.where(p[:, None] > p[None, :], BIG, 0.0).astype(np.float32)
    masktile = np.tile(masktri, (1, 4))
    ident = np.eye(128, dtype=np.float32)
    bd = np.zeros((128, 4, 8), np.float32)
    for c in range(4):
        for pp in range(128):
            bd[pp, c, 2 * c + pp // 64] = 1.0
    negdiag = np.zeros((8, 1024), np.float32)
    for half in range(2):
        for h in range(4):
            negdiag[half * 4 + h, half * 512 + h * 128:half * 512 + (h + 1) * 128] = -1.0
    indg = np.zeros((8, 512), np.float16)
    for g in range(4):
        for d in range(128):
            indg[2 * g + d // 64, g * 128 + d] = 1.0
    return masktile, ident, bd, negdiag, indg


def make_in_maps(key_in, query_in, value_in, Wk, bk, Wq, bq, Wv, bv, Wo, bo):
    masktile, ident, bd, negdiag, indg = _consts()
    maps = []
    for core in range(NC):
        b, hh = core // 2, core % 2
        sl = slice(hh * 512, (hh + 1) * 512)
        maps.append({
            "xq": np.ascontiguousarray(np.asarray(query_in[b])).astype(np.float16),
            "xkT": np.ascontiguousarray(np.asarray(key_in[b]).T).astype(np.float16),
            "xvT": np.ascontiguousarray(np.asarray(value_in[b]).T).astype(np.float16),
            "wq": np.ascontiguousarray(np.asarray(Wq)[:, sl]).astype(np.float16),
            "wkT": np.ascontiguousarray(np.asarray(Wk)[:, sl].T).astype(np.float16),
            "wv": np.ascontiguousarray(np.asarray(Wv)[:, sl]).astype(np.float16),
            "wo": np.ascontiguousarray(np.asarray(Wo)[sl, :]).astype(np.float16),
            "qbT": (S * np.asarray(bq)[sl]).reshape(4, 128).T.astype(np.float32).copy(),
            "masktile": masktile, "ident": ident, "bdmask": bd,
            "negdiag": negdiag, "indg": indg,
        })
    return maps


def run(inputs, trace=False):
    nc = _get_nc()
    in_maps = make_in_maps(**inputs)
    try:
        res = run_bass_kernel_spmd(nc, in_maps, list(range(NC)), trace=trace)
    except ModuleNotFoundError:
        os.environ["BASS_NEVER_TRACE"] = "1"
        res = run_bass_kernel_spmd(nc, in_maps, list(range(NC)), trace=False)
    Wo = np.asarray(inputs["Wo"], np.float32)
    extra = (np.asarray(inputs["bv"], np.float32) @ Wo
             + np.asarray(inputs["bo"], np.float32)).astype(np.float32)
    out = np.empty((4, S, D), np.float32)
    for b in range(4):
        out[b] = (res.results[2 * b]["outT"].T.astype(np.float32)
                  + res.results[2 * b + 1]["outT"].T.astype(np.float32)
                  + extra)
    return out, res


def kernel(**inputs):
    out, _ = run(inputs, trace=False)
    return out


# revision 32
# speedup vs baseline: 1.0159x; 1.0159x over previous
"""Trainium2 Bass kernel for nn_Attention_59347858096503 (v2).

Reference computation (per batch b, head H):
    k = split_heads(key_in @ Wk + bk); q = ...; v = ...
    qsum = q.sum(axis=positions)                      # (b,H,D)
    scores[s] = k[s] . qsum                           # per-key score (no q dep!)
    attn[q,k] = softmax over keys k<=q of scores[k]   # prefix softmax
    ctx[q] = sum_k attn[q,k] v[k];  out = ctx @ Wo + bo

Attention reduces to a prefix-softmax-weighted running average of v computed
blockwise (128 keys/queries per block) with a running (N, Z) carry at the
prefix-max scale m (prefix max of scores).

v2 design vs v1:
  - fp16 host staging of every tensor (score chain float accumulates on
    device stay f32/PSUM): halves HBM traffic; PE matmuls hit 1 cy/row.
  - G = mask + m[t] - s[k] built by PE rank-1 matmuls (mask via identity
    contraction, [ones; scoresT] stacked A against [m; -blockdiag] B),
    consumed by ACT exp directly from PSUM -> F fp16 in SBUF.
  - CT produced TRANSPOSED: matmul(CT[d,t], V[k,d-head], F[k,t]) so ctx
    needs no PE transposes / PSUM round trips before the out-projection.
  - Z rides V as a 65th ones-column (Vaug); per-block per-head carry rows
    NZ = [N | Z] at scale m[last] come from F's last column in 2 matmuls.
  - alpha (carry rescale) = one ACT exp per block on [8,128] with
    per-partition bias.
  - 1/Z broadcast to ctx layout via PE (indg x zrT), ctx = CT * zrb in one
    DVE op per block writing fp16 ctxT.
  - out-proj reads ctxT tiles directly; results DMA straight from PSUM to
    HBM (f32), skipping oT SBUF copies.

Sharding: batch 4-way x head-half 2-way = 8 cores; host sums the two f32
partials per batch. bk drops (softmax shift); bq via qsum; bv/bo on host.
"""
import os
import sys

sys.path.insert(0, "/opt/trn_rl_repo")

import numpy as np
import concourse.bass as bass
import concourse.tile as tile
from concourse import bacc, mybir
from concourse.bass_utils import run_bass_kernel_spmd

F32 = mybir.dt.float32
F32R = mybir.dt.float32r
F16 = mybir.dt.float16
AF = mybir.ActivationFunctionType
ALU = mybir.AluOpType

S = 1024
D = 1024
NH = 8        # heads per core
HD = 64       # head dim
NB = 8        # key/query blocks of 128
NC = 8        # cores
BIG = 30000.0


def _r(ap):
    return ap.bitcast(F32R)


def build(num_devices=NC):
    nc = bacc.Bacc(None, target_bir_lowering=False, debug=False,
                   num_devices=num_devices)

    xq_d = nc.dram_tensor("xq", [S, D], F16, kind="ExternalInput")
    xkT_d = nc.dram_tensor("xkT", [D, S], F16, kind="ExternalInput")
    xvT_d = nc.dram_tensor("xvT", [D, S], F16, kind="ExternalInput")
    wq_d = nc.dram_tensor("wq", [D, 512], F16, kind="ExternalInput")
    wkT_d = nc.dram_tensor("wkT", [512, D], F16, kind="ExternalInput")
    wv_d = nc.dram_tensor("wv", [D, 512], F16, kind="ExternalInput")
    wo_d = nc.dram_tensor("wo", [512, D], F16, kind="ExternalInput")
    qb_d = nc.dram_tensor("qbT", [128, 4], F32, kind="ExternalInput")
    mask_d = nc.dram_tensor("masktile", [128, 512], F32, kind="ExternalInput")
    id_d = nc.dram_tensor("ident", [128, 128], F32, kind="ExternalInput")
    bd_d = nc.dram_tensor("bdmask", [128, 4, 8], F32, kind="ExternalInput")
    nd_d = nc.dram_tensor("negdiag", [8, 1024], F32, kind="ExternalInput")
    ig_d = nc.dram_tensor("indg", [8, 512], F16, kind="ExternalInput")
    outT_d = nc.dram_tensor("outT", [D, S], F16, kind="ExternalOutput")


    with tile.TileContext(nc) as tc:
        with (
            tc.tile_pool(name="const", bufs=1) as cpool,
            tc.tile_pool(name="wts", bufs=1) as wpool,
            tc.tile_pool(name="big", bufs=1) as bpool,
            tc.tile_pool(name="sc", bufs=1) as scpool,
            tc.tile_pool(name="xs", bufs=3) as xspool,
            tc.tile_pool(name="sm", bufs=2) as smpool,
            tc.tile_pool(name="psG", bufs=2, space="PSUM") as psG,
            tc.tile_pool(name="psC1", bufs=1, space="PSUM") as psC1,
            tc.tile_pool(name="psC2", bufs=2, space="PSUM") as psC2,
            tc.tile_pool(name="psZ", bufs=2, space="PSUM") as psZ,
            tc.tile_pool(name="psM", bufs=1, space="PSUM") as psM,
        ):
            # ---- constants ----
            ident = cpool.tile([128, 128], F32)
            nc.sync.dma_start(ident[:], id_d.ap())
            identr = cpool.tile([128, 128], F32R)
            nc.gpsimd.dma_start(out=identr[:], in_=id_d.ap())
            masktile = cpool.tile([128, 512], F32R)
            nc.gpsimd.dma_start(out=masktile[:], in_=mask_d.ap())
            negdiag = cpool.tile([8, 1024], F32R)
            nc.gpsimd.dma_start(out=negdiag[:], in_=nd_d.ap())
            indg = cpool.tile([8, 512], F16)
            nc.sync.dma_start(indg[:], ig_d.ap())
            qb_sb = cpool.tile([128, 4], F32)
            nc.sync.dma_start(qb_sb[:], qb_d.ap())
            bdmask = cpool.tile([128, 4, 8], F32)
            nc.sync.dma_start(bdmask[:], bd_d.ap())
            ones_col = cpool.tile([128, 1], F16)
            nc.vector.memset(ones_col[:], 1.0)
            ones_f32 = cpool.tile([1, 128], F32)
            nc.vector.memset(ones_f32[:], 1.0)
            ones_row = cpool.tile([1, 128], F32R)
            nc.gpsimd.dma_start(out=ones_row[:], in_=ones_f32[:])

            # m_flat[0, b, :] = per-block m rows (flattened by SWDGE later)
            m_flat = scpool.tile([1, NB, 1024], F32R, name="m_flat")

            # ---- weights for the score chain first (critical path) ----
            wq_sb = wpool.tile([128, 8, 512], F16)
            nc.sync.dma_start(wq_sb[:], wq_d.ap().rearrange("(j p) f -> p j f", p=128))

            # ---- xq stream (s-major) -> PE column sums xqsumT[f,fc] ----
            xqsumT_ps = psM.tile([128, 512], F32, tag="m1", name="xqsumT_ps")[:, 0:8]
            xts = []
            for j in range(8):
                xt = xspool.tile([128, 1024], F16, tag="xq", bufs=8)
                nc.sync.dma_start(xt[:], xq_d.ap()[j * 128:(j + 1) * 128, :])
                xts.append(xt)
            # consecutive matmuls per accumulation group (interleaved open
            # groups in one PSUM bank lose writes)
            for fc in range(8):
                for j in range(8):
                    nc.tensor.matmul(xqsumT_ps[:, fc:fc + 1],
                                     xts[j][:, fc * 128:(fc + 1) * 128], ones_col[:],
                                     start=(j == 0), stop=(j == 7))
            xqsumT = scpool.tile([128, 8], F16)
            nc.vector.tensor_copy(xqsumT[:], xqsumT_ps[:])

            xv_ap = xvT_d.ap().rearrange("(j p) s -> p j s", p=128)

            # ---- qsumT[dm, c] columns = sum_f wq[f, dm] xqsum[f] + S*bq ----
            qsumT_ps = psM.tile([128, 512], F32, tag="m1", name="qsumT_ps")[:, 0:4]
            for dc in range(4):
                for fc in range(8):
                    nc.tensor.matmul(qsumT_ps[:, dc:dc + 1],
                                     wq_sb[:, fc, dc * 128:(dc + 1) * 128],
                                     xqsumT[:, fc:fc + 1],
                                     start=(fc == 0), stop=(fc == 7))
            qt_sb = scpool.tile([128, 4], F32)
            nc.vector.tensor_add(qt_sb[:], qsumT_ps[:], qb_sb[:])

            # ---- qsblk[f, c, h] = qsum[c*128+f] * (head(c,f) == h), fp16 ----
            qsblk = scpool.tile([128, 4, 8], F16)
            for c in range(4):
                nc.vector.tensor_scalar_mul(qsblk[:, c, :], bdmask[:, c, :],
                                            qt_sb[:, c:c + 1])

            wkT_sb = wpool.tile([128, 4, 1024], F16)
            nc.sync.dma_start(wkT_sb[:], wkT_d.ap().rearrange("(c p) m -> p c m", p=128))

            # ---- u[dm, h] = sum_f wkT[f, dm] qsblk[f, h] ----
            u_sb = scpool.tile([128, 8, 8], F16)
            for i in range(8):
                u_ps = psM.tile([128, 512], F32, tag="m1", name="u_ps")[:, 0:8]
                for c in range(4):
                    nc.tensor.matmul(u_ps[:], wkT_sb[:, c, i * 128:(i + 1) * 128],
                                     qsblk[:, c, :], start=(c == 0), stop=(c == 3))
                nc.vector.tensor_copy(u_sb[:, i, :], u_ps[:])

            # ---- scoresT (8, 1024) = u.T @ xkT (xk streamed) ----
            scoresT_psA = psG.tile([128, 512], F32, tag="g", name="scoresT_psA")[0:8, :]
            scoresT_psB = psG.tile([128, 512], F32, tag="g", name="scoresT_psB")[0:8, :]
            sc_half = (scoresT_psA, scoresT_psB)
            for i in range(8):
                xt = xspool.tile([128, 1024], F16, tag="xk", bufs=4)
                nc.sync.dma_start(xt[:], xkT_d.ap()[i * 128:(i + 1) * 128, :])
                for half in range(2):
                    nc.tensor.matmul(
                        sc_half[half][:],
                        u_sb[:, i, :], xt[:, half * 512:(half + 1) * 512],
                        start=(i == 0), stop=(i == 7))

            # ---- V-chain loads AFTER the xk stream on the same (sync) queue:
            # the score chain owns the DMA pipe first; V-proj JIT eats slices.
            wv_sb = wpool.tile([128, 8, 512], F16)
            xvT_sb = bpool.tile([128, 8, 1024], F16, name="xvT_sb")
            nc.sync.dma_start(wv_sb[:], wv_d.ap().rearrange("(j p) f -> p j f", p=128))
            for vs in range(4):
                nc.sync.dma_start(xvT_sb[:, :, vs * 256:(vs + 1) * 256],
                                  xv_ap[:, :, vs * 256:(vs + 1) * 256])
            wo_sb = wpool.tile([128, 4, 1024], F16)
            nc.sync.dma_start(wo_sb[:], wo_d.ap().rearrange("(c p) m -> p c m", p=128))

            # ---- scoresT rows in SBUF (lhsT for the G s-broadcast) ----
            scA = scpool.tile([8, 1024], F32R, name="scA")
            nc.vector.tensor_copy(scA[:, 0:512], scoresT_psA[:])
            nc.vector.tensor_copy(scA[:, 512:1024], scoresT_psB[:])

            # ---- chained prefix-max m per block + mS row DMAs + alpha ----
            m_all = scpool.tile([8, 1024], F32R, name="m_all")
            alpha8 = scpool.tile([8, NB, 128], F16, name="alpha8")
            alpha = scpool.tile([1, NB, 1024], F16, name="alpha")
            for blk in range(NB):
                sl = slice(blk * 128, (blk + 1) * 128)
                init = -3.0e38 if blk == 0 else m_all[:, blk * 128 - 1:blk * 128]
                nc.vector.tensor_tensor_scan(m_all[:, sl], scA[:, sl],
                                             scA[:, sl], init,
                                             ALU.max, ALU.max)
                nc.gpsimd.dma_start(out=m_flat[0:1, blk, :], in_=m_all[:, sl])
                if blk > 0:
                    prev = blk * 128 - 1
                    nc.scalar.activation(alpha8[:, blk, :], m_all[:, sl],
                                         AF.Exp, scale=-1.0,
                                         bias=m_all[:, prev:prev + 1])
                    nc.gpsimd.dma_start(out=alpha[0:1, blk, :], in_=alpha8[:, blk, :])

            # ---- V_sb: per-block projected V with a ones 65th column ----
            V_sb = bpool.tile([128, NB, NH, 65], F16, name="V_sb")
            nc.vector.memset(V_sb[:], 1.0)

            def emit_vproj(blk):
                V_ps = psZ.tile([128, 512], F32, tag="z", name="V_ps")
                for j in range(8):
                    nc.tensor.matmul(
                        V_ps[:],
                        xvT_sb[:, j, blk * 128:(blk + 1) * 128],
                        wv_sb[:, j, :],
                        start=(j == 0), stop=(j == 7))
                if blk % 2 == 0:
                    nc.vector.tensor_copy(V_sb[:, blk, :, 0:64], V_ps[:])
                else:
                    nc.scalar.copy(V_sb[:, blk, :, 0:64], V_ps[:])

            # ---- attention blocks ----
            F_sb = bpool.tile([128, 2, 1024], F16, name="F_sb")
            ctxT_sb = bpool.tile([128, 4, 1024], F16, name="ctxT_sb")
            NZprev = None

            def emit_pair(q):
                # out-proj for s-blocks 2q, 2q+1 -> outT[j, q*256:(q+1)*256]
                ssl = slice(q * 256, (q + 1) * 256)
                oT = smpool.tile([128, 8, 256], F16, tag="oT", name="oT", bufs=2)
                for jj in range(4):
                    O_ps = psZ.tile([128, 512], F32, tag="z", name="O_ps")
                    for j2 in range(2):
                        jb = jj * 2 + j2
                        for c in range(4):
                            nc.tensor.matmul(
                                O_ps[:, j2 * 256:(j2 + 1) * 256],
                                wo_sb[:, c, jb * 128:(jb + 1) * 128],
                                ctxT_sb[:, c, ssl],
                                start=(c == 0), stop=(c == 3))
                    dst = oT[:, jj * 2:jj * 2 + 2, :]
                    srcv = O_ps[:].rearrange("p (j2 s) -> p j2 s", j2=2)
                    if jj % 2 == 0:
                        nc.vector.tensor_copy(dst, srcv)
                    else:
                        nc.scalar.copy(dst, srcv)
                nc.sync.dma_start(
                    outT_d.ap().rearrange("(jb p) s -> p jb s", p=128)[:, :, ssl],
                    oT[:])

            for blk in range(NB):
                sl = slice(blk * 128, (blk + 1) * 128)
                par = blk % 2
                if blk == 0:
                    emit_vproj(0)
                    emit_vproj(1)
                elif blk < NB - 1:
                    emit_vproj(blk + 1)
                # --- G = mask + m[t] - s[k] per half; F = exp(-G) fp16 ---
                G_h = []
                for half in range(2):
                    hsl = slice(half * 512, (half + 1) * 512)
                    G_ps = psG.tile([128, 512], F32, tag="g", name="G_ps")
                    nc.tensor.matmul(G_ps[:], identr[:], masktile[:],
                                     start=True, stop=False)
                    nc.tensor.matmul(G_ps[:], ones_row[:],
                                     m_flat[0:1, blk, hsl],
                                     start=False, stop=False)
                    nc.tensor.matmul(G_ps[:], scA[:, sl],
                                     negdiag[:, hsl],
                                     start=False, stop=True)
                    G_h.append(G_ps)
                # carries into CT/CZ first (ready before F) then F-dependent.
                # CZNZ bank: CZ [128 t, 8 h] at cols 0:8; NZ rows (N|Z at
                # scale m[last]) at partitions 0 (heads 0-3) / 64 (heads 4-7),
                # cols 128:388.
                CZNZ = psC1.tile([128, 512], F32, tag="cz", name="CZNZ")
                CZ = CZNZ[:, 0:8]
                CT = psC2.tile([128, 512], F32, tag="ct", name="CT")

                def nz_view(t, h):
                    p0 = 64 * (h // 4)
                    c0 = 128 + (h % 4) * 65
                    return t[p0:p0 + 1, c0:c0 + 65]

                for half in range(2):
                    hsl = slice(half * 512, (half + 1) * 512)
                    nc.scalar.activation(F_sb[:, par, hsl], G_h[half][:],
                                         AF.Exp, scale=-1.0)
                first = blk == 0
                for h in range(NH):
                    g, dpar = h // 2, h % 2
                    Fh = F_sb[:, par, h * 128:(h + 1) * 128]
                    nc.tensor.matmul(
                        CT[dpar * 64:(dpar + 1) * 64, g * 128:(g + 1) * 128],
                        V_sb[:, blk, h, 0:64], Fh,
                        start=True, stop=first)
                    nc.tensor.matmul(
                        CZ[:, h:h + 1], Fh, ones_col[:],
                        start=True, stop=first)
                    if blk > 0:
                        nc.tensor.matmul(
                            CT[dpar * 64:(dpar + 1) * 64, g * 128:(g + 1) * 128],
                            NZprev[0:1, (h // 4) * 260 + (h % 4) * 65:(h // 4) * 260 + (h % 4) * 65 + 64],
                            alpha[0:1, blk, h * 128:(h + 1) * 128],
                            start=False, stop=True)
                        nc.tensor.matmul(
                            CZ[:, h:h + 1],
                            alpha[0:1, blk, h * 128:(h + 1) * 128],
                            NZprev[0:1, (h // 4) * 260 + (h % 4) * 65 + 64:(h // 4) * 260 + (h % 4) * 65 + 65],
                            start=False, stop=True)
                    if blk < NB - 1:
                        nc.tensor.matmul(
                            nz_view(CZNZ, h),
                            F_sb[:, par, h * 128 + 127:h * 128 + 128],
                            V_sb[:, blk, h, 0:65],
                            start=True, stop=first)
                        if blk > 0:
                            nc.tensor.matmul(
                                nz_view(CZNZ, h),
                                alpha[0:1, blk, h * 128 + 127:h * 128 + 128],
                                NZprev[0:1, (h // 4) * 260 + (h % 4) * 65:(h // 4) * 260 + (h % 4) * 65 + 65],
                                start=False, stop=True)
                # 1/Z [128,8] -> PE transpose -> zrT [8,128] fp16
                zr_all = smpool.tile([128, 8], F32, tag="zra", name="zr_all")
                nc.vector.reciprocal(zr_all[:], CZ[:])
                zrt_ps = psM.tile([128, 512], F32, tag="m1", name="zrt_ps")
                nc.tensor.transpose(zrt_ps[0:8, 0:128], zr_all[:], ident[:])
                zrT = smpool.tile([8, 128], F16, tag="zrt", name="zrT")
                nc.vector.tensor_copy(zrT[:], zrt_ps[0:8, 0:128])
                if blk < NB - 1:
                    NZnew = smpool.tile([1, 520], F16, tag="nzsb", name="NZsb")
                    nc.vector.tensor_copy(NZnew[0:1, 0:260], CZNZ[0:1, 128:388])
                    nc.vector.tensor_copy(NZnew[0:1, 260:520], CZNZ[64:65, 128:388])
                    NZprev = NZnew
                zrb = psM.tile([128, 512], F32, tag="m1", name="zrb")
                for g in range(4):
                    nc.tensor.matmul(zrb[:, g * 128:(g + 1) * 128],
                                     indg[:, g * 128:(g + 1) * 128],
                                     zrT[:], start=True, stop=True)
                zrb_sb = smpool.tile([128, 512], F16, tag="zrb_sb", name="zrb_sb")
                nc.scalar.copy(zrb_sb[:], zrb[:])
                nc.vector.tensor_tensor(
                    ctxT_sb[:, :, sl],
                    CT[:].rearrange("p (c s) -> p c s", c=4),
                    zrb_sb[:].rearrange("p (c s) -> p c s", c=4), ALU.mult)

                if blk % 2 == 1:
                    emit_pair(blk // 2)
            


    nc.compile()
    return nc


_NC_CACHE = {}


def _get_nc():
    if "nc" not in _NC_CACHE:
        _NC_CACHE["nc"] = build()
    return _NC_CACHE["nc"]


def _consts():
    p = np.arange(128)
    masktri = np.where(p[:, None] > p[None, :], BIG, 0.0).astype(np.float32)
    masktile = np.tile(masktri, (1, 4))
    ident = np.eye(128, dtype=np.float32)
    bd = np.zeros((128, 4, 8), np.float32)
    for c in range(4):
        for pp in range(128):
            bd[pp, c, 2 * c + pp // 64] = 1.0
    negdiag = np.zeros((8, 1024), np.float32)
    for half in range(2):
        for h in range(4):
            negdiag[half * 4 + h, half * 512 + h * 128:half * 512 + (h + 1) * 128] = -1.0
    indg = np.zeros((8, 512), np.float16)
    for g in range(4):
        for d in range(128):
            indg[2 * g + d // 64, g * 128 + d] = 1.0
    return masktile, ident, bd, negdiag, indg


def make_in_maps(key_in, query_in, value_in, Wk, bk, Wq, bq, Wv, bv, Wo, bo):
    masktile, ident, bd, negdiag, indg = _consts()
    maps = []
    for core in range(NC):
        b, hh = core // 2, core % 2
        sl = slice(hh * 512, (hh + 1) * 512)
        maps.append({
            "xq": np.ascontiguousarray(np.asarray(query_in[b])).astype(np.float16),
            "xkT": np.ascontiguousarray(np.asarray(key_in[b]).T).astype(np.float16),
            "xvT": np.ascontiguousarray(np.asarray(value_in[b]).T).astype(np.float16),
            "wq": np.ascontiguousarray(np.asarray(Wq)[:, sl]).astype(np.float16),
            "wkT": np.ascontiguousarray(np.asarray(Wk)[:, sl].T).astype(np.float16),
            "wv": np.ascontiguousarray(np.asarray(Wv)[:, sl]).astype(np.float16),
            "wo": np.ascontiguousarray(np.asarray(Wo)[sl, :]).astype(np.float16),
            "qbT": (S * np.asarray(bq)[sl]).reshape(4, 128).T.astype(np.float32).copy(),
            "masktile": masktile, "ident": ident, "bdmask": bd,
            "negdiag": negdiag, "indg": indg,
        })
    return maps


def run(inputs, trace=False):
    nc = _get_nc()
    in_maps = make_in_maps(**inputs)
    try:
        res = run_bass_kernel_spmd(nc, in_maps, list(range(NC)), trace=trace)
    except ModuleNotFoundError:
        os.environ["BASS_NEVER_TRACE"] = "1"
        res = run_bass_kernel_spmd(nc, in_maps, list(range(NC)), trace=False)
    Wo = np.asarray(inputs["Wo"], np.float32)
    extra = (np.asarray(inputs["bv"], np.float32) @ Wo
             + np.asarray(inputs["bo"], np.float32)).astype(np.float32)
    out = np.empty((4, S, D), np.float32)
    for b in range(4):
        out[b] = (res.results[2 * b]["outT"].T.astype(np.float32)
                  + res.results[2 * b + 1]["outT"].T.astype(np.float32)
                  + extra)
    return out, res


def kernel(**inputs):
    out, _ = run(inputs, trace=False)
    return out
